# revision 3
# baseline (speedup 1.0000x reference)
"""Trainium2 Bass kernel for batched single-head attention.

Problem: x[8, 4096, 512] fp32, Wq/Wk/Wv[512, 256], bq/bk/bv[256].
  Q = x@Wq + bq ; K = x@Wk + bk ; V = x@Wv + bv
  out = softmax(Q K^T / sqrt(256)) V          -> [8, 4096, 256]

Sharding: data-parallel over batch. 8 batch elements -> 8 NeuronCores,
one full attention per core, no collectives.

Per-core algorithm (all fp32, PSUM accumulation):
  1. xT = x.T via PE matmul-with-identity (regular matmul, lhsT = x chunk).
  2. QT/KT [e, s] = W.T @ xT (weights stationary, N=512 moving), bias added
     on the PSUM->SBUF copy via per-partition activation bias.
  3. V [s, e] natural layout (xT chunks stationary), bias via a rank-1
     (K=1) ones @ bv matmul into the same PSUM group. A ones column is
     appended to V so attn@V also yields softmax row sums for free.
  4. Per q-block of 512: scoresT [k, q] = KT.T @ QT block (PE), exp((.)/16)
     on ACT directly PSUM->SBUF (no max subtraction: scores ~ N(0,1), exp
     is fp32-safe), then out[q, 0:257] += PT_chunk.T @ Vext per k-chunk.
     Normalize with the row sums (col 256) on the way out.
"""

import sys

if "/opt/trn_rl_repo" not in sys.path:
    sys.path.insert(0, "/opt/trn_rl_repo")

import numpy as np

import concourse.bass as bass  # noqa: F401  (AP types reachable via tiles)
import concourse.mybir as mybir
import concourse.tile as tile
from concourse import bacc
from concourse.bass_utils import run_bass_kernel_spmd
from concourse.masks import make_identity

FP32 = mybir.dt.float32
AF = mybir.ActivationFunctionType

N_CORES = 8
B, S, DIN, D = 8, 4096, 512, 256
P = 128
S_TILES = S // P      # 32 s-tiles
DC = DIN // P         # 4 din chunks
ECH = D // P          # 2 e chunks
QB = 512              # q-block width (columns of scoresT)
N_QB = S // QB        # 8 q-blocks
VE = D + 1            # V columns + ones column = 257
VE_PAD = 260          # padded free extent for the Vext tile
SCALE = 0.0625        # 1/sqrt(256), exact in fp32


def build_program():
    nc = bacc.Bacc(
        "TRN2", target_bir_lowering=False, debug=False, num_devices=N_CORES
    )
    x_d = nc.dram_tensor("x", [S, DIN], FP32, kind="ExternalInput")
    wq_d = nc.dram_tensor("Wq", [DIN, D], FP32, kind="ExternalInput")
    bq_d = nc.dram_tensor("bq", [D], FP32, kind="ExternalInput")
    wk_d = nc.dram_tensor("Wk", [DIN, D], FP32, kind="ExternalInput")
    bk_d = nc.dram_tensor("bk", [D], FP32, kind="ExternalInput")
    wv_d = nc.dram_tensor("Wv", [DIN, D], FP32, kind="ExternalInput")
    bv_d = nc.dram_tensor("bv", [D], FP32, kind="ExternalInput")
    out_d = nc.dram_tensor("out", [S, D], FP32, kind="ExternalOutput")

    with tile.TileContext(nc) as tc:
        with (
            tc.tile_pool(name="const", bufs=1) as constp,
            tc.tile_pool(name="big", bufs=1) as bigp,
        ):
            ident = constp.tile([P, P], FP32)
            make_identity(nc, ident[:])
            ones_row = constp.tile([1, P], FP32)
            nc.gpsimd.memset(ones_row[:], 1.0)

            # Weights: [128, 4, 256] with [:, c, :] = W[c*128:(c+1)*128, :]
            wq_sb = constp.tile([P, DC, D], FP32)
            wk_sb = constp.tile([P, DC, D], FP32)
            wv_sb = constp.tile([P, DC, D], FP32)
            nc.sync.dma_start(wq_sb[:], wq_d.rearrange("(c p) d -> p c d", p=P))
            nc.sync.dma_start(wk_sb[:], wk_d.rearrange("(c p) d -> p c d", p=P))
            nc.sync.dma_start(wv_sb[:], wv_d.rearrange("(c p) d -> p c d", p=P))
            # Per-partition bias layout for QT/KT: [:, c] = b[c*128:(c+1)*128]
            bqT = constp.tile([P, ECH], FP32)
            bkT = constp.tile([P, ECH], FP32)
            nc.sync.dma_start(bqT[:], bq_d.rearrange("(c p) -> p c", p=P))
            nc.sync.dma_start(bkT[:], bk_d.rearrange("(c p) -> p c", p=P))
            # Row layout for the V bias rank-1 update
            bv_row = constp.tile([1, D], FP32)
            nc.sync.dma_start(bv_row[:], bv_d.rearrange("(o d) -> o d", o=1))

            qt = bigp.tile([P, ECH, S], FP32)   # QT: [e-chunk part, ec, s]
            kt = bigp.tile([P, ECH, S], FP32)
            vext = bigp.tile([P, S_TILES, VE_PAD], FP32)  # V + ones col
            nc.gpsimd.memset(vext[:, :, D : D + 1], 1.0)

            # ---- Phase 1: transpose x -> xT; Phase 2: projections ----
            with tc.tile_pool(name="xTpool", bufs=1) as xtp:
                xt = xtp.tile([P, DC, S], FP32)  # xT: [din-chunk part, dc, s]
                with (
                    tc.tile_pool(name="xload", bufs=3) as xlp,
                    tc.tile_pool(name="tps", bufs=4, space="PSUM") as tpsp,
                ):
                    for st in range(S_TILES):
                        xtile = xlp.tile([P, DIN], FP32)
                        nc.sync.dma_start(
                            xtile[:], x_d[st * P : (st + 1) * P, :]
                        )
                        for dc in range(DC):
                            pst = tpsp.tile([P, P], FP32)
                            nc.tensor.matmul(
                                pst[:],
                                xtile[:, dc * P : (dc + 1) * P],
                                ident[:],
                                start=True,
                                stop=True,
                            )
                            nc.vector.tensor_copy(
                                xt[:, dc, st * P : (st + 1) * P], pst[:]
                            )

                with (
                    tc.tile_pool(name="pjq", bufs=3, space="PSUM") as pjq,
                    tc.tile_pool(name="pjv", bufs=3, space="PSUM") as pjv,
                ):
                    # QT / KT
                    for w_sb, bT, dst in ((wq_sb, bqT, qt), (wk_sb, bkT, kt)):
                        for ec in range(ECH):
                            for sb in range(N_QB):
                                ps = pjq.tile([P, QB], FP32)
                                for dc in range(DC):
                                    nc.tensor.matmul(
                                        ps[:],
                                        w_sb[:, dc, ec * P : (ec + 1) * P],
                                        xt[:, dc, sb * QB : (sb + 1) * QB],
                                        start=(dc == 0),
                                        stop=(dc == DC - 1),
                                    )
                                nc.scalar.activation(
                                    dst[:, ec, sb * QB : (sb + 1) * QB],
                                    ps[:],
                                    AF.Identity,
                                    bias=bT[:, ec : ec + 1],
                                )
                    # V natural layout + bias rank-1
                    for st in range(S_TILES):
                        psv = pjv.tile([P, D], FP32)
                        for dc in range(DC):
                            nc.tensor.matmul(
                                psv[:],
                                xt[:, dc, st * P : (st + 1) * P],
                                wv_sb[:, dc, :],
                                start=(dc == 0),
                                stop=False,
                            )
                        nc.tensor.matmul(
                            psv[:],
                            ones_row[:],
                            bv_row[:],
                            start=False,
                            stop=True,
                        )
                        nc.vector.tensor_copy(vext[:, st, 0:D], psv[:])

            # ---- Phase 3: attention ----
            with (
                tc.tile_pool(name="ptp", bufs=3) as ptp,
                tc.tile_pool(name="accp", bufs=5, space="PSUM") as accp,
                tc.tile_pool(name="scp", bufs=2, space="PSUM") as scp,
                tc.tile_pool(name="outp", bufs=4) as outp,
                tc.tile_pool(name="nrmp", bufs=4) as nrmp,
            ):
                for qb in range(N_QB):
                    accs = [
                        accp.tile([P, VE], FP32, name="acc", tag="acc")
                        for _ in range(QB // P)
                    ]
                    for kt_i in range(S_TILES):
                        pss = scp.tile([P, QB], FP32)
                        for ec in range(ECH):
                            nc.tensor.matmul(
                                pss[:],
                                kt[:, ec, kt_i * P : (kt_i + 1) * P],
                                qt[:, ec, qb * QB : (qb + 1) * QB],
                                start=(ec == 0),
                                stop=(ec == ECH - 1),
                            )
                        ptt = ptp.tile([P, QB], FP32)
                        nc.scalar.activation(
                            ptt[:], pss[:], AF.Exp, scale=SCALE
                        )
                        for j in range(QB // P):
                            nc.tensor.matmul(
                                accs[j][:],
                                ptt[:, j * P : (j + 1) * P],
                                vext[:, kt_i, 0:VE],
                                start=(kt_i == 0),
                                stop=(kt_i == S_TILES - 1),
                            )
                    for j in range(QB // P):
                        rc = nrmp.tile([P, 1], FP32)
                        nc.vector.reciprocal(rc[:], accs[j][:, D : D + 1])
                        ot = outp.tile([P, D], FP32)
                        nc.scalar.mul(ot[:], accs[j][:, 0:D], rc[:])
                        row = (qb * (QB // P) + j) * P
                        nc.sync.dma_start(out_d[row : row + P, :], ot[:])

    nc.compile()
    return nc


_NC_CACHE = []


def _get_nc():
    if not _NC_CACHE:
        _NC_CACHE.append(build_program())
    return _NC_CACHE[0]


def kernel(**inputs) -> np.ndarray:
    x = np.ascontiguousarray(np.asarray(inputs["x"], dtype=np.float32))
    w = {
        k: np.ascontiguousarray(np.asarray(inputs[k], dtype=np.float32))
        for k in ("Wq", "bq", "Wk", "bk", "Wv", "bv")
    }
    nc = _get_nc()
    in_maps = [{"x": x[b], **w} for b in range(B)]
    res = run_bass_kernel_spmd(nc, in_maps, list(range(N_CORES)))
    return np.stack([res.results[b]["out"] for b in range(B)], axis=0)


# revision 4
# speedup vs baseline: 2.8509x; 2.8509x over previous
"""Trainium2 Bass kernel for batched single-head attention.

Problem: x[8, 4096, 512] fp32, Wq/Wk/Wv[512, 256], bq/bk/bv[256].
  Q = x@Wq + bq ; K = x@Wk + bk ; V = x@Wv + bv
  out = softmax(Q K^T / sqrt(256)) V          -> [8, 4096, 256]

Sharding: data-parallel over batch. 8 batch elements -> 8 NeuronCores,
one full attention per core, no collectives.

All matmuls run in bf16 with fp32 PSUM accumulation (fp32 matmuls on
TRN2 lower to an FP32HI/FP32LO pass pair AND stream the moving operand
at half rate — measured ~4x slower than bf16). Biases are added in fp32
on the PSUM->SBUF copy; softmax row sums / normalization stay fp32.

Per-core algorithm:
  1. xT = x.T via PE matmul-with-identity (x arrives pre-cast to bf16).
  2. QT/KT [e, s] = W.T @ xT (weights stationary, N=512 moving), bias
     added on the PSUM->SBUF copy via per-partition activation bias.
  3. V [s, e] natural layout (xT chunks stationary), bias via a rank-1
     (K=1) ones @ bv matmul into the same PSUM group. A fp32->bf16 ones
     column is appended to V so attn@V also yields softmax row sums.
  4. Per q-block of 512: scoresT [k, q] = KT.T @ QT block (PE), exp((.)/16)
     on ACT directly PSUM->SBUF (no max subtraction: scores ~ N(0,1), exp
     is fp32-safe), then out[q, 0:257] += PT_chunk.T @ Vext per k-chunk.
     Normalize with the fp32 row sums (col 256) on the way out.
"""

import sys

if "/opt/trn_rl_repo" not in sys.path:
    sys.path.insert(0, "/opt/trn_rl_repo")

import ml_dtypes
import numpy as np

import concourse.bass as bass  # noqa: F401
import concourse.mybir as mybir
import concourse.tile as tile
from concourse import bacc
from concourse.bass_utils import run_bass_kernel_spmd
from concourse.masks import make_identity

FP32 = mybir.dt.float32
BF16 = mybir.dt.bfloat16
AF = mybir.ActivationFunctionType

N_CORES = 8
B, S, DIN, D = 8, 4096, 512, 256
P = 128
S_TILES = S // P      # 32 s-tiles
DC = DIN // P         # 4 din chunks
ECH = D // P          # 2 e chunks
QB = 512              # q-block width (columns of scoresT)
N_QB = S // QB        # 8 q-blocks
VE = D + 1            # V columns + ones column = 257
VE_PAD = 260          # padded free extent for the Vext tile
SCALE = 0.0625        # 1/sqrt(256), exact in fp32


def build_program():
    nc = bacc.Bacc(
        "TRN2", target_bir_lowering=False, debug=False, num_devices=N_CORES
    )
    x_d = nc.dram_tensor("x", [S, DIN], BF16, kind="ExternalInput")
    wq_d = nc.dram_tensor("Wq", [DIN, D], BF16, kind="ExternalInput")
    bq_d = nc.dram_tensor("bq", [D], FP32, kind="ExternalInput")
    wk_d = nc.dram_tensor("Wk", [DIN, D], BF16, kind="ExternalInput")
    bk_d = nc.dram_tensor("bk", [D], FP32, kind="ExternalInput")
    wv_d = nc.dram_tensor("Wv", [DIN, D], BF16, kind="ExternalInput")
    bv_d = nc.dram_tensor("bv", [D], BF16, kind="ExternalInput")
    out_d = nc.dram_tensor("out", [S, D], FP32, kind="ExternalOutput")

    with tile.TileContext(nc) as tc:
        with (
            tc.tile_pool(name="const", bufs=1) as constp,
            tc.tile_pool(name="big", bufs=1) as bigp,
        ):
            ident = constp.tile([P, P], BF16)
            make_identity(nc, ident[:])
            ones_row = constp.tile([1, P], BF16)
            nc.gpsimd.memset(ones_row[:], 1.0)

            # Weights: [128, 4, 256] with [:, c, :] = W[c*128:(c+1)*128, :]
            wq_sb = constp.tile([P, DC, D], BF16)
            wk_sb = constp.tile([P, DC, D], BF16)
            wv_sb = constp.tile([P, DC, D], BF16)
            nc.sync.dma_start(wq_sb[:], wq_d.rearrange("(c p) d -> p c d", p=P))
            nc.sync.dma_start(wk_sb[:], wk_d.rearrange("(c p) d -> p c d", p=P))
            nc.sync.dma_start(wv_sb[:], wv_d.rearrange("(c p) d -> p c d", p=P))
            # Per-partition bias layout for QT/KT: [:, c] = b[c*128:(c+1)*128]
            bqT = constp.tile([P, ECH], FP32)
            bkT = constp.tile([P, ECH], FP32)
            nc.sync.dma_start(bqT[:], bq_d.rearrange("(c p) -> p c", p=P))
            nc.sync.dma_start(bkT[:], bk_d.rearrange("(c p) -> p c", p=P))
            # Row layout for the V bias rank-1 update
            bv_row = constp.tile([1, D], BF16)
            nc.sync.dma_start(bv_row[:], bv_d.rearrange("(o d) -> o d", o=1))

            qt = bigp.tile([P, ECH, S], BF16)   # QT: [e-chunk part, ec, s]
            kt = bigp.tile([P, ECH, S], BF16)
            vext = bigp.tile([P, S_TILES, VE_PAD], BF16)  # V + ones col
            nc.gpsimd.memset(vext[:, :, D : D + 1], 1.0)

            # ---- Phase 1: transpose x -> xT; Phase 2: projections ----
            with tc.tile_pool(name="xTpool", bufs=1) as xtp:
                xt = xtp.tile([P, DC, S], BF16)  # xT: [din-chunk part, dc, s]
                with (
                    tc.tile_pool(name="xload", bufs=3) as xlp,
                    tc.tile_pool(name="tps", bufs=4, space="PSUM") as tpsp,
                ):
                    for st in range(S_TILES):
                        xtile = xlp.tile([P, DIN], BF16)
                        nc.sync.dma_start(
                            xtile[:], x_d[st * P : (st + 1) * P, :]
                        )
                        for dc in range(DC):
                            pst = tpsp.tile([P, P], FP32)
                            nc.tensor.matmul(
                                pst[:],
                                xtile[:, dc * P : (dc + 1) * P],
                                ident[:],
                                start=True,
                                stop=True,
                            )
                            nc.vector.tensor_copy(
                                xt[:, dc, st * P : (st + 1) * P], pst[:]
                            )

                with (
                    tc.tile_pool(name="pjq", bufs=3, space="PSUM") as pjq,
                    tc.tile_pool(name="pjv", bufs=3, space="PSUM") as pjv,
                ):
                    # QT / KT
                    for w_sb, bT, dst in ((wq_sb, bqT, qt), (wk_sb, bkT, kt)):
                        for ec in range(ECH):
                            for sb in range(N_QB):
                                ps = pjq.tile([P, QB], FP32)
                                for dc in range(DC):
                                    nc.tensor.matmul(
                                        ps[:],
                                        w_sb[:, dc, ec * P : (ec + 1) * P],
                                        xt[:, dc, sb * QB : (sb + 1) * QB],
                                        start=(dc == 0),
                                        stop=(dc == DC - 1),
                                    )
                                nc.scalar.activation(
                                    dst[:, ec, sb * QB : (sb + 1) * QB],
                                    ps[:],
                                    AF.Identity,
                                    bias=bT[:, ec : ec + 1],
                                )
                    # V natural layout + bias rank-1
                    for st in range(S_TILES):
                        psv = pjv.tile([P, D], FP32)
                        for dc in range(DC):
                            nc.tensor.matmul(
                                psv[:],
                                xt[:, dc, st * P : (st + 1) * P],
                                wv_sb[:, dc, :],
                                start=(dc == 0),
                                stop=False,
                            )
                        nc.tensor.matmul(
                            psv[:],
                            ones_row[:],
                            bv_row[:],
                            start=False,
                            stop=True,
                        )
                        nc.vector.tensor_copy(vext[:, st, 0:D], psv[:])

            # ---- Phase 3: attention ----
            with (
                tc.tile_pool(name="ptp", bufs=3) as ptp,
                tc.tile_pool(name="accp", bufs=5, space="PSUM") as accp,
                tc.tile_pool(name="scp", bufs=2, space="PSUM") as scp,
                tc.tile_pool(name="outp", bufs=4) as outp,
                tc.tile_pool(name="nrmp", bufs=4) as nrmp,
            ):
                for qb in range(N_QB):
                    accs = [
                        accp.tile([P, VE], FP32, name="acc", tag="acc")
                        for _ in range(QB // P)
                    ]
                    for kt_i in range(S_TILES):
                        pss = scp.tile([P, QB], FP32)
                        for ec in range(ECH):
                            nc.tensor.matmul(
                                pss[:],
                                kt[:, ec, kt_i * P : (kt_i + 1) * P],
                                qt[:, ec, qb * QB : (qb + 1) * QB],
                                start=(ec == 0),
                                stop=(ec == ECH - 1),
                            )
                        ptt = ptp.tile([P, QB], BF16)
                        nc.scalar.activation(
                            ptt[:], pss[:], AF.Exp, scale=SCALE
                        )
                        for j in range(QB // P):
                            nc.tensor.matmul(
                                accs[j][:],
                                ptt[:, j * P : (j + 1) * P],
                                vext[:, kt_i, 0:VE],
                                start=(kt_i == 0),
                                stop=(kt_i == S_TILES - 1),
                            )
                    for j in range(QB // P):
                        rc = nrmp.tile([P, 1], FP32)
                        nc.vector.reciprocal(rc[:], accs[j][:, D : D + 1])
                        ot = outp.tile([P, D], FP32)
                        nc.scalar.mul(ot[:], accs[j][:, 0:D], rc[:])
                        row = (qb * (QB // P) + j) * P
                        nc.sync.dma_start(out_d[row : row + P, :], ot[:])

    nc.compile()
    return nc


_NC_CACHE = []


def _get_nc():
    if not _NC_CACHE:
        _NC_CACHE.append(build_program())
    return _NC_CACHE[0]


def kernel(**inputs) -> np.ndarray:
    BF = ml_dtypes.bfloat16
    x = np.ascontiguousarray(np.asarray(inputs["x"]).astype(BF))
    w = {}
    for k in ("Wq", "Wk", "Wv", "bv"):
        w[k] = np.ascontiguousarray(np.asarray(inputs[k]).astype(BF))
    for k in ("bq", "bk"):
        w[k] = np.ascontiguousarray(np.asarray(inputs[k]).astype(np.float32))
    nc = _get_nc()
    in_maps = [{"x": x[b], **w} for b in range(B)]
    res = run_bass_kernel_spmd(nc, in_maps, list(range(N_CORES)))
    return np.stack([res.results[b]["out"] for b in range(B)], axis=0)


# revision 6
# speedup vs baseline: 3.8914x; 1.3650x over previous
"""Trainium2 Bass kernel for batched single-head attention.

Problem: x[8, 4096, 512] fp32, Wq/Wk/Wv[512, 256], bq/bk/bv[256].
  Q = x@Wq + bq ; K = x@Wk + bk ; V = x@Wv + bv
  out = softmax(Q K^T / sqrt(256)) V          -> [8, 4096, 256]

Sharding: data-parallel over batch. 8 batch elements -> 8 NeuronCores,
one full attention per core, no collectives.

All matmuls run in bf16 with fp32 PSUM accumulation (fp32 matmuls on
TRN2 lower to an FP32HI/FP32LO pass pair AND stream the moving operand
at half rate — measured ~4x slower than bf16). Biases are added in fp32
on the PSUM->SBUF copy; softmax row sums / normalization stay fp32.

Per-core algorithm:
  1. xT = x.T via PE matmul-with-identity (x arrives pre-cast to bf16).
  2. QT/KT [e, s] = W.T @ xT (weights stationary, N=512 moving), bias
     added on the PSUM->SBUF copy via per-partition activation bias.
  3. V [s, e] natural layout (xT chunks stationary), bias via a rank-1
     (K=1) ones @ bv matmul into the same PSUM group. A fp32->bf16 ones
     column is appended to V so attn@V also yields softmax row sums.
  4. Per q-block of 512: scoresT [k, q] = KT.T @ QT block (PE), exp((.)/16)
     on ACT directly PSUM->SBUF (no max subtraction: scores ~ N(0,1), exp
     is fp32-safe), then out[q, 0:257] += PT_chunk.T @ Vext per k-chunk.
     Normalize with the fp32 row sums (col 256) on the way out.
"""

import sys

if "/opt/trn_rl_repo" not in sys.path:
    sys.path.insert(0, "/opt/trn_rl_repo")

import ml_dtypes
import numpy as np

import concourse.bass as bass  # noqa: F401
import concourse.mybir as mybir
import concourse.tile as tile
from concourse import bacc
from concourse.bass_utils import run_bass_kernel_spmd
from concourse.masks import make_identity

FP32 = mybir.dt.float32
BF16 = mybir.dt.bfloat16
AF = mybir.ActivationFunctionType

N_CORES = 8
B, S, DIN, D = 8, 4096, 512, 256
P = 128
S_TILES = S // P      # 32 s-tiles
DC = DIN // P         # 4 din chunks
ECH = D // P          # 2 e chunks
QB = 512              # q-block width (columns of scoresT)
N_QB = S // QB        # 8 q-blocks
VE = D + 1            # V columns + ones column = 257
VE_PAD = 260          # padded free extent for the Vext tile
SCALE = 0.0625        # 1/sqrt(256), exact in fp32


def build_program():
    nc = bacc.Bacc(
        "TRN2", target_bir_lowering=False, debug=False, num_devices=N_CORES
    )
    x_d = nc.dram_tensor("x", [S, DIN], BF16, kind="ExternalInput")
    wq_d = nc.dram_tensor("Wq", [DIN, D], BF16, kind="ExternalInput")
    bq_d = nc.dram_tensor("bq", [D], FP32, kind="ExternalInput")
    wk_d = nc.dram_tensor("Wk", [DIN, D], BF16, kind="ExternalInput")
    bk_d = nc.dram_tensor("bk", [D], FP32, kind="ExternalInput")
    wv_d = nc.dram_tensor("Wv", [DIN, D], BF16, kind="ExternalInput")
    bv_d = nc.dram_tensor("bv", [D], BF16, kind="ExternalInput")
    out_d = nc.dram_tensor("out", [S, D], FP32, kind="ExternalOutput")

    with tile.TileContext(nc) as tc:
        with (
            tc.tile_pool(name="const", bufs=1) as constp,
            tc.tile_pool(name="big", bufs=1) as bigp,
        ):
            ident = constp.tile([P, P], BF16)
            make_identity(nc, ident[:])
            ones_row = constp.tile([1, P], BF16)
            nc.gpsimd.memset(ones_row[:], 1.0)

            # Weights: [128, 4, 256] with [:, c, :] = W[c*128:(c+1)*128, :]
            wq_sb = constp.tile([P, DC, D], BF16)
            wk_sb = constp.tile([P, DC, D], BF16)
            wv_sb = constp.tile([P, DC, D], BF16)
            nc.sync.dma_start(wq_sb[:], wq_d.rearrange("(c p) d -> p c d", p=P))
            nc.sync.dma_start(wk_sb[:], wk_d.rearrange("(c p) d -> p c d", p=P))
            nc.sync.dma_start(wv_sb[:], wv_d.rearrange("(c p) d -> p c d", p=P))
            # Per-partition bias layout for QT/KT: [:, c] = b[c*128:(c+1)*128]
            bqT = constp.tile([P, ECH], FP32)
            bkT = constp.tile([P, ECH], FP32)
            nc.sync.dma_start(bqT[:], bq_d.rearrange("(c p) -> p c", p=P))
            nc.sync.dma_start(bkT[:], bk_d.rearrange("(c p) -> p c", p=P))
            # Row layout for the V bias rank-1 update
            bv_row = constp.tile([1, D], BF16)
            nc.sync.dma_start(bv_row[:], bv_d.rearrange("(o d) -> o d", o=1))

            qt = bigp.tile([P, ECH, S], BF16)   # QT: [e-chunk part, ec, s]
            kt = bigp.tile([P, ECH, S], BF16)
            vext = bigp.tile([P, S_TILES, VE_PAD], BF16)  # V + ones col
            nc.gpsimd.memset(vext[:, :, D : D + 1], 1.0)

            # ---- Phase 1: transpose x -> xT; Phase 2: projections ----
            with tc.tile_pool(name="xTpool", bufs=1) as xtp:
                xt = xtp.tile([P, DC, S], BF16)  # xT: [din-chunk part, dc, s]
                with (
                    tc.tile_pool(name="xload", bufs=3) as xlp,
                    tc.tile_pool(name="tps", bufs=3, space="PSUM") as tpsp,
                ):
                    for st in range(S_TILES):
                        xtile = xlp.tile([P, DIN], BF16)
                        nc.sync.dma_start(
                            xtile[:], x_d[st * P : (st + 1) * P, :]
                        )
                        # 4 transposed chunks into one full PSUM bank,
                        # then a single strided cast to SBUF.
                        pst = tpsp.tile([P, DIN], FP32)
                        for dc in range(DC):
                            nc.tensor.matmul(
                                pst[:, dc * P : (dc + 1) * P],
                                xtile[:, dc * P : (dc + 1) * P],
                                ident[:],
                                start=True,
                                stop=True,
                            )
                        src = pst[:].rearrange("p (c f) -> p c f", c=DC)
                        dstv = xt[:, :, st * P : (st + 1) * P]
                        if st % 2 == 0:
                            nc.vector.tensor_copy(dstv, src)
                        else:
                            nc.scalar.copy(dstv, src)

                with (
                    tc.tile_pool(name="pjq", bufs=3, space="PSUM") as pjq,
                    tc.tile_pool(name="pjv", bufs=3, space="PSUM") as pjv,
                ):
                    # QT / KT
                    for w_sb, bT, dst in ((wq_sb, bqT, qt), (wk_sb, bkT, kt)):
                        for ec in range(ECH):
                            for sb in range(N_QB):
                                ps = pjq.tile([P, QB], FP32)
                                for dc in range(DC):
                                    nc.tensor.matmul(
                                        ps[:],
                                        w_sb[:, dc, ec * P : (ec + 1) * P],
                                        xt[:, dc, sb * QB : (sb + 1) * QB],
                                        start=(dc == 0),
                                        stop=(dc == DC - 1),
                                    )
                                nc.scalar.activation(
                                    dst[:, ec, sb * QB : (sb + 1) * QB],
                                    ps[:],
                                    AF.Identity,
                                    bias=bT[:, ec : ec + 1],
                                )
                    # V natural layout + bias rank-1
                    for st in range(S_TILES):
                        psv = pjv.tile([P, D], FP32)
                        for dc in range(DC):
                            nc.tensor.matmul(
                                psv[:],
                                xt[:, dc, st * P : (st + 1) * P],
                                wv_sb[:, dc, :],
                                start=(dc == 0),
                                stop=False,
                            )
                        nc.tensor.matmul(
                            psv[:],
                            ones_row[:],
                            bv_row[:],
                            start=False,
                            stop=True,
                        )
                        nc.vector.tensor_copy(vext[:, st, 0:D], psv[:])

            # ---- Phase 3: attention (software-pipelined: scores run
            # LOOKAHEAD k-tiles ahead of attn@V so the PE never waits on
            # the ACT exp latency) ----
            LOOKAHEAD = 2
            with (
                tc.tile_pool(name="ptp", bufs=4) as ptp,
                tc.tile_pool(name="accp", bufs=5, space="PSUM") as accp,
                tc.tile_pool(name="scp", bufs=3, space="PSUM") as scp,
                tc.tile_pool(name="outp", bufs=4) as outp,
                tc.tile_pool(name="nrmp", bufs=4) as nrmp,
            ):
                for qb in range(N_QB):
                    accs = [
                        accp.tile([P, VE], FP32, name="acc", tag="acc")
                        for _ in range(QB // P)
                    ]
                    ptts = {}
                    for step in range(S_TILES + LOOKAHEAD):
                        if step < S_TILES:
                            kt_i = step
                            pss = scp.tile([P, QB], FP32)
                            for ec in range(ECH):
                                nc.tensor.matmul(
                                    pss[:],
                                    kt[:, ec, kt_i * P : (kt_i + 1) * P],
                                    qt[:, ec, qb * QB : (qb + 1) * QB],
                                    start=(ec == 0),
                                    stop=(ec == ECH - 1),
                                )
                            ptt = ptp.tile([P, QB], BF16)
                            nc.scalar.activation(
                                ptt[:], pss[:], AF.Exp, scale=SCALE
                            )
                            ptts[kt_i] = ptt
                        av = step - LOOKAHEAD
                        if av >= 0:
                            pav = ptts.pop(av)
                            for j in range(QB // P):
                                nc.tensor.matmul(
                                    accs[j][:],
                                    pav[:, j * P : (j + 1) * P],
                                    vext[:, av, 0:VE],
                                    start=(av == 0),
                                    stop=(av == S_TILES - 1),
                                )
                    for j in range(QB // P):
                        rc = nrmp.tile([P, 1], FP32)
                        nc.vector.reciprocal(rc[:], accs[j][:, D : D + 1])
                        ot = outp.tile([P, D], FP32)
                        nc.vector.tensor_scalar_mul(
                            ot[:], accs[j][:, 0:D], rc[:]
                        )
                        row = (qb * (QB // P) + j) * P
                        nc.sync.dma_start(out_d[row : row + P, :], ot[:])

    nc.compile()
    return nc


_NC_CACHE = []


def _get_nc():
    if not _NC_CACHE:
        _NC_CACHE.append(build_program())
    return _NC_CACHE[0]


def kernel(**inputs) -> np.ndarray:
    BF = ml_dtypes.bfloat16
    x = np.ascontiguousarray(np.asarray(inputs["x"]).astype(BF))
    w = {}
    for k in ("Wq", "Wk", "Wv", "bv"):
        w[k] = np.ascontiguousarray(np.asarray(inputs[k]).astype(BF))
    for k in ("bq", "bk"):
        w[k] = np.ascontiguousarray(np.asarray(inputs[k]).astype(np.float32))
    nc = _get_nc()
    in_maps = [{"x": x[b], **w} for b in range(B)]
    res = run_bass_kernel_spmd(nc, in_maps, list(range(N_CORES)))
    return np.stack([res.results[b]["out"] for b in range(B)], axis=0)
